# revision 49
# baseline (speedup 1.0000x reference)
"""Trainium2 Bass kernel for nn_DecoderForLarge (sparse attention decoder).

Shapes (hardcoded): B=64, N=1000, G=500, H=256. 8 NeuronCores, batch-sharded
(8 batches per core). All matmuls run as single-term float32r (12-bit
mantissa): measured end-to-end error ~5e-3 absmax-relative vs the fp32
reference (gate 2e-2).

Design notes:
  - emb loaded interleaved (n = 8p + c) so per-partition DMA extents are
    8KB-contiguous; used directly as f32r via bitcast (PE rounds).
  - mask additive term is clamped to -2^26 (bf16): applied *before* the
    tanh clip, so visited nodes saturate to exactly -10; softmax runs
    without max-subtraction (clip bounds exp to [e^-10, e^10]).
  - last-node embeddings and dists rows are gathered via SWDGE indirect
    DMA (bf16-cast inline); all gather indices are preloaded in one DMA.
  - transposes (emb^T, lastemb^T, maskprob^T) run as plain matmuls
    against identity (counts as PE-busy for the HAM clock gate).
  - pooled/lastT matmuls run bf16 (32-bit moving operands stream at
    half rate through the PE); emb/mask values exact or ~2^-9 error,
    well in budget. Output stored bf16.
  - software pipeline per period: head_dma(b+2) | head_compute(b+1) |
    tail(b) | head_fq(b+1) — loads lead consumers by ~1.5 batch periods
    and every PSUM->SBUF copy gets a full PE phase to land before its
    LDWEIGHTS consumer.
"""

import sys

for _p in ("/opt/trn_rl_repo", "/root/.axon_site/_ro/trn_rl_repo"):
    if _p not in sys.path:
        sys.path.append(_p)

import numpy as np

import concourse.bass as bass
import concourse.mybir as mybir
import concourse.tile as tile
from concourse.masks import make_identity
from concourse.bass_utils import run_bass_kernel_spmd

F32 = mybir.dt.float32
F32R = mybir.dt.float32r
BF16 = mybir.dt.bfloat16
I32 = mybir.dt.int32

B, N, G, H = 64, 1000, 500, 256
NCORES = 8
NB = B // NCORES          # batches per core
GC = 125                  # G chunk (4 chunks of 125)
NGC = G // GC
NCH = 8                   # N interleave chunks (n = 8p + c), p=0..124
TANH_CLIP = 10.0
INV_SQRT_H = float(1.0 / np.sqrt(np.float32(H)))
NEG_INV_SQRT_2 = float(-np.float32(1.0 / np.sqrt(2.0)))
MASK_NEG = -float(2.0 ** 26)   # additive mask; exact in bf16/f32r


def _split_excess_waits(nc, maxw=1):
    # This walrus build rejects >1 semaphore wait per instruction
    # (CoreV3 setupSyncWait). Move extras onto preceding same-engine NoOps.
    for f in nc.m.functions:
        for bb in f.blocks:
            newlist = []
            for ins in bb.instructions:
                si = ins.sync_info
                if si is not None and si.on_wait is not None and len(si.on_wait) > maxw:
                    waits = list(si.on_wait)
                    extra, keep = waits[:-maxw], waits[-maxw:]
                    for i in range(0, len(extra), maxw):
                        nop = mybir.InstNoOp(name=f"{ins.name}-ws{i}", ins=[], outs=[])
                        nop.engine = ins.engine
                        nop.sync_info = mybir.SyncInfo(on_wait=extra[i:i + maxw], on_update=[])
                        newlist.append(nop)
                    ins.sync_info = mybir.SyncInfo(on_wait=keep, on_update=list(si.on_update or []))
                newlist.append(ins)
            bb.instructions[:] = newlist


def build_nc(nb=NB):
    nc = bass.Bass("TRN2", target_bir_lowering=False, debug=False,
                   num_swdge_queues=4)
    Alu = mybir.AluOpType
    Act = mybir.ActivationFunctionType

    def _on_queue(inst, qn):
        # indirect_dma_start pins queue="qPoolDynamic"; rotate across the 4
        # SWDGE queues to spread descriptors over more SDMA engines
        if qn:
            inst.ins.queue = f"qPoolDynamic{qn}"
        return inst

    emb_e = nc.dram_tensor("emb", [nb, N, H], F32R, kind="ExternalInput").ap()
    dist_e = nc.dram_tensor("dists", [nb, N, N], F32, kind="ExternalInput").ap()
    ln_e = nc.dram_tensor("last_node", [nb, G], I32, kind="ExternalInput").ap()
    mask_e = nc.dram_tensor("mask", [nb, G, N], F32, kind="ExternalInput").ap()
    w_e = {}
    for w in ("wlf", "wv", "wg"):
        w_e[w] = nc.dram_tensor(w, [H, H], F32R, kind="ExternalInput").ap()
    out_e = nc.dram_tensor("out", [nb, G, N], BF16, kind="ExternalOutput").ap()

    dist_flat = dist_e.rearrange("b n m -> (b n) m")
    emb_flat = emb_e.rearrange("b n h -> (b n) h").bitcast(F32)

    with tile.TileContext(nc) as tc:
        import contextlib
        with contextlib.ExitStack() as ctx:
            const = ctx.enter_context(tc.tile_pool(name="const", bufs=1))
            mrawp = ctx.enter_context(tc.tile_pool(name="mrawp", bufs=3))
            mprob = ctx.enter_context(tc.tile_pool(name="mprob", bufs=2))
            etp = ctx.enter_context(tc.tile_pool(name="etp", bufs=2))
            lep = ctx.enter_context(tc.tile_pool(name="lep", bufs=3))
            distp = ctx.enter_context(tc.tile_pool(name="distp", bufs=3))
            lrp = ctx.enter_context(tc.tile_pool(name="lrp", bufs=3))
            headq = ctx.enter_context(tc.tile_pool(name="headq", bufs=1))
            fqp = ctx.enter_context(tc.tile_pool(name="fqp", bufs=2))
            smp = ctx.enter_context(tc.tile_pool(name="smp", bufs=2))
            outp = ctx.enter_context(tc.tile_pool(name="outp", bufs=4))
            tiny = ctx.enter_context(tc.tile_pool(name="tiny", bufs=4))
            ps_tp = ctx.enter_context(tc.tile_pool(name="ps_tp", bufs=3, space="PSUM"))
            ps_pq = ctx.enter_context(tc.tile_pool(name="ps_pq", bufs=2, space="PSUM"))
            ps_sc = ctx.enter_context(tc.tile_pool(name="ps_sc", bufs=3, space="PSUM"))

            # ---- constants ----
            identf = const.tile([128, 128], F32, name="identf")
            make_identity(nc, identf[:])
            identr = const.tile([128, 128], F32R, name="identr")
            nc.vector.tensor_copy(out=identr[:], in_=identf[:])
            identb = const.tile([128, 128], BF16, name="identb")
            nc.vector.tensor_copy(out=identb[:], in_=identf[:])
            ones_f = const.tile([128, 4], F32, name="ones_f")
            nc.gpsimd.memset(ones_f[:], 1.0)
            ones_row = const.tile([1, G], BF16, name="ones_row")
            nc.vector.tensor_copy(out=ones_row[:], in_=ones_f[0:1, 0:1].to_broadcast([1, G]))
            wt = {}

            def load_weights():
                # emitted after head_dma(0) so batch 0's loads go first on
                # the sync ring (shaves the cold-start PE stall)
                for w, ap_ in w_e.items():
                    t = const.tile([128, 2, H], F32R, name=w)
                    nc.sync.dma_start(out=t[:], in_=ap_.rearrange("(c p) o -> p c o", p=128))
                    tb = const.tile([128, 2, H], BF16, name=w + "b")
                    nc.vector.tensor_copy(out=tb[:], in_=t.bitcast(F32)[:])
                    wt[w] = tb

            # persistent double-buffered emb (pad rows 125:127 zeroed once so
            # matmul K=126 stays even (f32r rule); DMA rewrites rows 0:125)
            emb_f = []
            for i in range(3):
                t = const.tile([128, NCH, H], F32R, name=f"emb_f{i}")
                nc.gpsimd.memset(t.bitcast(F32)[96:128, :, :], 0.0)
                emb_f.append(t)
            # persistent maskT (bf16: mask values are exact; bf16 moving
            # operands stream 2x faster than 32-bit through the PE):
            # pad rows + mean-pool ones columns written once
            maskT = const.tile([128, NCH, G + 4], BF16, name="maskT")
            nc.gpsimd.memset(maskT[96:128, :, :], 0.0)
            for c in range(NCH):
                nc.vector.tensor_copy(out=maskT[:GC, c, G:G + 4], in_=ones_f[:GC, :])
            # bf16 copy of emb for the pooled contraction (triple-buffered;
            # pad rows stay zero because emb_f pads are zero)
            emb_b = [const.tile([128, NCH, H], BF16, name=f"emb_b{i}")
                     for i in range(3)]

            # all batches' gather indices preloaded once: removes the
            # per-batch idx-DMA -> add -> gather latency chain
            idx_all = const.tile([GC, nb, NGC], I32, name="idx_all")
            idxg_all = const.tile([GC, nb, NGC], I32, name="idxg_all")

            def load_idx():
                nc.sync.dma_start(
                    out=idx_all[:], in_=ln_e.rearrange("b (c p) -> p b c", p=GC))
                for bb in range(nb):
                    nc.vector.tensor_scalar_add(
                        idxg_all[:, bb, :], idx_all[:, bb, :], bb * N)

            def head_dma(b, emb_loaded=False):
                st = {}
                idxg = idxg_all[:, b, :]

                # ---- gathers + loads, in order of consumption: lastemb
                # (lastT, early), mask (maskT, mid), dists (tail only, last)
                # per-gc tile names: pool bufs rotate per allocation of a
                # name, so a shared name would collide within one batch
                lastemb = lep.tile([GC, NGC, H], BF16, name="lastemb")
                for gc in range(NGC):
                    _on_queue(nc.gpsimd.indirect_dma_start(
                        out=lastemb[:, gc, :], out_offset=None, in_=emb_flat,
                        in_offset=bass.IndirectOffsetOnAxis(ap=idxg[:, gc:gc + 1], axis=0)),
                        gc)
                ef = emb_f[b % 3]
                if not emb_loaded:
                    nc.sync.dma_start(
                        out=ef[0:GC, :, :],
                        in_=emb_e[b].rearrange("(p c) h -> p c h", c=NCH))
                # mask rides SWDGE with inline f32->bf16 cast (cast DMA is
                # SWDGE-only); rotated across queues, offset from the gathers
                mraw_t = []
                for gc in range(NGC):
                    mr = mrawp.tile([GC, N], BF16, name=f"mraw{gc}")
                    _on_queue(nc.gpsimd.dma_start(
                        out=mr[:], in_=mask_e[b, gc * GC:(gc + 1) * GC, :]),
                        (gc + 2) % 4)
                    mraw_t.append(mr)
                dist_t = []
                for gc in range(NGC):
                    dt_ = distp.tile([GC, N], BF16, name=f"dist{gc}")
                    _on_queue(nc.gpsimd.indirect_dma_start(
                        out=dt_[:], out_offset=None, in_=dist_flat,
                        in_offset=bass.IndirectOffsetOnAxis(ap=idxg[:, gc:gc + 1], axis=0)),
                        gc)
                    dist_t.append(dt_)

                st.update(ef=ef, dist_t=dist_t, mraw_t=mraw_t, lastemb=lastemb)
                return st

            def head_compute(b, st):
                ef, mraw_t = st["ef"], st["mraw_t"]
                ef_r = ef[:, :, :]
                ebf = emb_b[b % 3]
                nc.scalar.copy(out=ebf[:], in_=ef.bitcast(F32)[:, :, :])

                # ---- embT + lastT via PE transposes; grouped back-to-back
                # (transposes pipeline ~40-100ns apart) ----
                embT = etp.tile([128, 2, N], F32R, name="embT")
                lastT = headq.tile([128, 2, G], BF16, name="lastT")

                def embT_block(hc, half):
                    cs = range(4 * half, 4 * half + 4)
                    ptp = ps_tp.tile([128, 512], F32, name="tpr", tag="tp")
                    for j, c in enumerate(cs):
                        nc.tensor.matmul(
                            out=ptp[:, j * 128:j * 128 + 126],
                            lhsT=ef_r[:126, c, hc * 128:(hc + 1) * 128],
                            rhs=identr[:126, :126],
                            skip_group_check=True)
                    # scatter block columns back to natural n order
                    # (n = 8q + c): out free dims (c, q) strides (1, 8)
                    ov = embT[:, hc, :].rearrange("p (q c) -> p c q", c=NCH)
                    nc.scalar.copy(
                        out=ov[:, 4 * half:4 * half + 4, :],
                        in_=ptp[:, :].rearrange("p (a q) -> p a q", a=4)[:, :, 0:GC])

                def lastT_block(hc):
                    # transpose gathered last-node embeddings [125,128] blocks
                    lptp = ps_tp.tile([128, 504], F32, name="tpf", tag="tp")
                    for gc in range(NGC):
                        nc.tensor.matmul(
                            out=lptp[:, gc * 126:gc * 126 + GC],
                            lhsT=st["lastemb"][:, gc, hc * 128:(hc + 1) * 128],
                            rhs=identb[:GC, :GC],
                            skip_group_check=True)
                    nc.vector.tensor_copy(
                        out=lastT[:, hc, :].rearrange("p (a g) -> p a g", a=NGC),
                        in_=lptp[:, :].rearrange("p (a g) -> p a g", a=NGC)[:, :, 0:GC])

                for hc in range(2):
                    embT_block(hc, 0)
                    embT_block(hc, 1)
                    lastT_block(hc)

                # ---- maskprob = max(mask, -2^26), bf16 in/out (2x DVE rate) ----
                maskprob = mprob.tile([GC, NGC, N], BF16, name="maskprob")
                for gc in range(NGC):
                    nc.vector.tensor_scalar_max(maskprob[:, gc, :], mraw_t[gc][:], MASK_NEG)

                # ---- maskT transposes interleaved with the pooled^T
                # accumulation, staggered one n-chunk so the PSUM->SBUF copy
                # of chunk c hides behind the transposes of chunk c+1 ----
                pooled = headq.tile([128, 2, G + 1], BF16, name="pooled")
                pp = [ps_pq.tile([128, G + 4], F32, name=f"pp{hc}", tag="pq")
                      for hc in range(2)]
                mp_il = maskprob[:, :, :].rearrange("p a (q c) -> p a c q", c=NCH)

                def maskT_block(c):
                    ptp = ps_tp.tile([128, 504], F32, name="tpb", tag="tp")
                    for gc in range(NGC):
                        nc.tensor.matmul(
                            out=ptp[:GC, gc * 126:(gc + 1) * 126],
                            lhsT=mp_il[:, gc, c, :],
                            rhs=identb[:GC, :126],
                            skip_group_check=True)
                    nc.vector.tensor_copy(
                        out=maskT[:GC, c, 0:G].rearrange("p (a g) -> p a g", a=NGC),
                        in_=ptp[:GC, :].rearrange("p (a g) -> p a g", a=NGC)[:, :, 0:GC])

                def pooled_block(c):
                    for hc in range(2):
                        nc.tensor.matmul(
                            out=pp[hc][:, :G + 4],
                            lhsT=ebf[:126, c, hc * 128:(hc + 1) * 128],
                            rhs=maskT[:126, c, :],
                            start=(c == 0), stop=(c == NCH - 1))

                for half in range(2):
                    for c in range(4 * half, 4 * half + 4):
                        maskT_block(c)
                    for c in range(4 * half, 4 * half + 4):
                        pooled_block(c)
                for hc in range(2):
                    nc.vector.tensor_copy(out=pooled[:, hc, :], in_=pp[hc][:, :G + 1])

                # ---- q_graph^T row: qg[1, H] ----
                qg_ps = ps_pq.tile([1, H], F32, name="qg", tag="pq")
                for kc in range(2):
                    nc.tensor.matmul(
                        out=qg_ps[:, :], lhsT=pooled[:, kc, G:G + 1],
                        rhs=wt["wg"][:, kc, :],
                        start=(kc == 0), stop=(kc == 1))
                qg_row = tiny.tile([1, H], BF16, name="qg_row")
                nc.vector.tensor_copy(out=qg_row[:], in_=qg_ps[:, :])

                st.update(embT=embT, maskprob=maskprob, lastT=lastT,
                          pooled=pooled, qg_row=qg_row)
                return st

            def head_fq(b, st):
                lastT, pooled, qg_row = st["lastT"], st["pooled"], st["qg_row"]
                # ---- fq^T = Wlf@lastT + Wv@pooled + qg (rank-1 broadcast) ----
                fq = fqp.tile([128, 2, G], F32R, name="fq")
                for hc in range(2):
                    qp = ps_pq.tile([128, G], F32, name="qp", tag="pq")
                    mms = []
                    for kc in range(2):
                        mms.append((wt["wlf"][:, kc, hc * 128:(hc + 1) * 128],
                                    lastT[:, kc, :]))
                    for kc in range(2):
                        mms.append((wt["wv"][:, kc, hc * 128:(hc + 1) * 128],
                                    pooled[:, kc, 0:G]))
                    mms.append((qg_row[:1, hc * 128:(hc + 1) * 128], ones_row[:, :]))
                    for i, (wap, xap) in enumerate(mms):
                        nc.tensor.matmul(
                            out=qp[:, :G], lhsT=wap, rhs=xap,
                            start=(i == 0), stop=(i == len(mms) - 1))
                    nc.vector.tensor_copy(out=fq[:, hc, :], in_=qp[:, :G])

                st.update(fq=fq)
                return st

            def tail(b, st):
                fq, embT, dist_t = st["fq"], st["embT"], st["dist_t"]
                maskprob = st["maskprob"]
                for gc in range(NGC):
                    # one PSUM tile per 500-col half: a matmul output must stay
                    # inside a single 2KB PSUM bank
                    sc = [ps_sc.tile([GC, 500], F32, name="sc", tag="sc")
                          for _ in range(2)]
                    for nh in range(2):
                        for kc in range(2):
                            nc.tensor.matmul(
                                out=sc[nh][:, :],
                                lhsT=fq[:, kc, gc * GC:(gc + 1) * GC],
                                rhs=embT[:, kc, nh * 500:(nh + 1) * 500],
                                start=(kc == 0), stop=(kc == 1))
                    # z = score - dist/sqrt2 + maskprob (visited saturate at
                    # clip=-10 through the tanh; no max-subtraction needed)
                    z = smp.tile([GC, N], F32, name="z")
                    zm = smp.tile([GC, N], F32, name="zm")
                    for nh in range(2):
                        nc.vector.scalar_tensor_tensor(
                            out=z[:, nh * 500:(nh + 1) * 500],
                            in0=dist_t[gc][:, nh * 500:(nh + 1) * 500],
                            scalar=NEG_INV_SQRT_2,
                            in1=sc[nh][:, :], op0=Alu.mult, op1=Alu.add)
                    nc.vector.tensor_tensor(
                        out=zm[:], in0=z[:], in1=maskprob[:, gc, :], op=Alu.add)
                    t_ = z                                 # reuse
                    nc.scalar.activation(out=t_[:], in_=zm[:], func=Act.Tanh, scale=1.0)
                    e = zm                                 # reuse
                    s = tiny.tile([GC, 1], F32, name="s")
                    nc.scalar.activation(
                        out=e[:], in_=t_[:], func=Act.Exp,
                        scale=TANH_CLIP, accum_out=s[:, :1])
                    r = tiny.tile([GC, 1], F32, name="r")
                    nc.vector.reciprocal(out=r[:], in_=s[:, :1])
                    o = outp.tile([GC, N], BF16, name="o")
                    nc.scalar.activation(out=o[:], in_=e[:], func=Act.Copy,
                                         scale=r[:, :1])
                    nc.scalar.dma_start(
                        out=out_e[b, gc * GC:(gc + 1) * GC, :], in_=o[:])

            # software pipeline, per batch period:
            #   head_dma(b+1) | head_compute(b+1) | tail(b) | head_fq(b+1)
            # tail(b)'s score matmuls sit between producing pooled/qg(b+1)
            # and consuming them (head_fq), and head_compute(b+1) sits
            # between fq(b) production and score(b) consumption — every
            # PSUM->SBUF copy gets a full PE phase to land, so LDWEIGHTS
            # never stalls on the vector engine.
            ef0 = emb_f[0]
            nc.sync.dma_start(
                out=ef0[0:GC, :, :],
                in_=emb_e[0].rearrange("(p c) h -> p c h", c=NCH))
            load_idx()
            pend = [head_dma(0, emb_loaded=True)]
            load_weights()
            if nb > 1:
                pend.append(head_dma(1))
            st = head_fq(0, head_compute(0, pend.pop(0)))
            prev = st
            for b in range(nb):
                if b + 2 < nb:
                    pend.append(head_dma(b + 2))
                nxt = None
                if b + 1 < nb:
                    nxt = head_compute(b + 1, pend.pop(0))
                tail(b, prev)
                if nxt is not None:
                    prev = head_fq(b + 1, nxt)

    _split_excess_waits(nc)
    return nc


_NC_CACHE = {}


def _get_nc(nb=NB):
    if nb not in _NC_CACHE:
        _NC_CACHE[nb] = build_nc(nb)
    return _NC_CACHE[nb]


def _prep_weights(Wq_graph, Wq_first, Wq_last, W_visited):
    Wq_graph = np.asarray(Wq_graph, np.float32)
    Wq_first = np.asarray(Wq_first, np.float32)
    Wq_last = np.asarray(Wq_last, np.float32)
    W_visited = np.asarray(W_visited, np.float32)
    s_h = np.float32(INV_SQRT_H)
    out = {}
    out["wlf"] = ((Wq_last + Wq_first).T * s_h).astype(np.float32)
    # maskprob is -2^26 * visited; fold the sign and scale into W_visited
    out["wv"] = (W_visited.T * (-s_h / np.float32(N * (-MASK_NEG)))).astype(np.float32)
    out["wg"] = (Wq_graph.T * (s_h / np.float32(N))).astype(np.float32)
    return out


def kernel(embeddings, dists, last_node, group_ninf_mask,
           Wq_graph, Wq_first, Wq_last, W_visited, **_ignored):
    embeddings = np.ascontiguousarray(np.asarray(embeddings), np.float32)
    dists = np.ascontiguousarray(np.asarray(dists), np.float32)
    group_ninf_mask = np.ascontiguousarray(np.asarray(group_ninf_mask), np.float32)
    ln = np.ascontiguousarray(np.asarray(last_node)).astype(np.int32)
    w = _prep_weights(Wq_graph, Wq_first, Wq_last, W_visited)

    nc = _get_nc(NB)
    in_maps = []
    for c in range(NCORES):
        sl = slice(c * NB, (c + 1) * NB)
        m = dict(emb=embeddings[sl], dists=dists[sl],
                 last_node=ln[sl], mask=group_ninf_mask[sl])
        m.update(w)
        in_maps.append(m)
    res = run_bass_kernel_spmd(nc, in_maps, list(range(NCORES)))
    out = np.concatenate([np.asarray(res.results[c]["out"], np.float32)
                          for c in range(NCORES)], axis=0)
    return out


if __name__ == "__main__":
    # quick smoke test with random data
    rng = np.random.default_rng(0)
    emb = rng.standard_normal((B, N, H), dtype=np.float32)
    d = rng.random((B, N, N), dtype=np.float32)
    lnod = rng.integers(0, N, (B, G)).astype(np.int32)
    visited = rng.random((B, G, N)) < 0.3
    mask = np.where(visited, -np.inf, 0.0).astype(np.float32)
    s = 1.0 / np.sqrt(H)
    ws = [rng.standard_normal((H, H), dtype=np.float32) * s for _ in range(4)]
    o = kernel(emb, d, lnod, mask, *ws)
    print("out", o.shape, o.dtype, o.sum())
